# revision 7
# baseline (speedup 1.0000x reference)
"""Trainium2 Bass kernel for nn_LinearCritic (SimCLR-style loss scores).

Pipeline (reference): for each of z1,z2 [4096,2048]:
  X = z @ W1.T ; X = BN(X)*gamma+beta ; Y = relu(X) ; U = Y @ W2.T ; P = BN(U)
then cosine-similarity blocks between normalized projections form a
[8192, 8192] score matrix (diag of s00/s11 = -inf), targets = arange.

Sharding: batch rows split across 8 cores (512 rows of z1 + 512 of z2 each).
BatchNorm batch statistics are global -> two tiny AllReduces; the 128-d
normalized projections are AllGathered; each core then computes its
1024-row block of the output (memory-bound: 32 MB of the 256 MB output).

Matmuls run in float32r (tf32-like, 11-bit mantissa, 4x fp32 throughput);
everything else is fp32.
"""
import sys

sys.path.insert(0, "/opt/trn_rl_repo")

import numpy as np

import concourse.bass as bass
import concourse.bacc as bacc
import concourse.mybir as mybir
import concourse.tile as tile
from concourse.bass_utils import run_bass_kernel_spmd

NCORES = 8
N = 4096          # batch rows per z tensor
D = 2048          # hidden dim
P = 128           # projection dim
R = N // NCORES   # 512 rows per core per z
KT = D // 128     # 16 k/j tiles
TEMP_SCALE = 2.0  # 1/TEMPERATURE
BN_EPS = 1e-5

f32 = mybir.dt.float32
f32r = mybir.dt.float32r

_CACHE = {}


def _build(dbg=False):
    nc = bacc.Bacc("TRN2", target_bir_lowering=False, debug=False,
                   num_devices=NCORES)

    zT = nc.dram_tensor("zT", [D, 2 * R], f32r, kind="ExternalInput")
    W1c = nc.dram_tensor("W1c", [KT, D, 128], f32r, kind="ExternalInput")
    W2c = nc.dram_tensor("W2c", [D, 128], f32r, kind="ExternalInput")
    gb = nc.dram_tensor("gb", [128, 2 * KT], f32, kind="ExternalInput")

    out_top = nc.dram_tensor("out_top", [R, 2 * N], f32, kind="ExternalOutput")
    out_bot = nc.dram_tensor("out_bot", [R, 2 * N], f32, kind="ExternalOutput")

    ar1_in = nc.dram_tensor("ar1_in", [128, 4 * KT], f32, kind="Internal")
    ar1_out = nc.dram_tensor("ar1_out", [128, 4 * KT], f32, kind="Internal",
                             addr_space="Shared")
    ar2_in = nc.dram_tensor("ar2_in", [128, 4], f32, kind="Internal")
    ar2_out = nc.dram_tensor("ar2_out", [128, 4], f32, kind="Internal",
                             addr_space="Shared")
    ag_in = nc.dram_tensor("ag_in", [128, 2 * R], f32, kind="Internal")
    ag_out = nc.dram_tensor("ag_out", [NCORES * 128, 2 * R], f32,
                            kind="Internal", addr_space="Shared")

    dbg_t = {}
    if dbg:
        dbg_t["x0"] = nc.dram_tensor("dbg_x0", [128, 2 * R], f32, kind="ExternalOutput")
        dbg_t["stats1g"] = nc.dram_tensor("dbg_stats1g", [128, 4 * KT], f32, kind="ExternalOutput")
        dbg_t["scale1"] = nc.dram_tensor("dbg_scale1", [128, 2 * KT], f32, kind="ExternalOutput")
        dbg_t["shift1"] = nc.dram_tensor("dbg_shift1", [128, 2 * KT], f32, kind="ExternalOutput")
        dbg_t["y0"] = nc.dram_tensor("dbg_y0", [128, 2 * R], f32, kind="ExternalOutput")
        dbg_t["u0"] = nc.dram_tensor("dbg_u0", [128, R], f32, kind="ExternalOutput")
        dbg_t["stats2g"] = nc.dram_tensor("dbg_stats2g", [128, 4], f32, kind="ExternalOutput")
        dbg_t["qloc"] = nc.dram_tensor("dbg_qloc", [128, 2 * R], f32, kind="ExternalOutput")
        dbg_t["qfull"] = nc.dram_tensor("dbg_qfull", [128, 2 * N], f32, kind="ExternalOutput")

    rg = [list(range(NCORES))]
    AF = mybir.ActivationFunctionType
    ALU = mybir.AluOpType

    with tile.TileContext(nc) as tc:
        with tc.tile_pool(name="small", bufs=1) as small, \
             tc.tile_pool(name="persist", bufs=1) as persist:
            # persistent small tiles
            stats1 = small.tile([128, 4 * KT], f32, tag="stats1")
            stats1_g = small.tile([128, 4 * KT], f32, tag="stats1g")
            gb_sb = small.tile([128, 2 * KT], f32, tag="gb")
            scale1 = small.tile([128, 2 * KT], f32, tag="scale1")
            shift1 = small.tile([128, 2 * KT], f32, tag="shift1")
            stats2 = small.tile([128, 4], f32, tag="stats2")
            stats2_g = small.tile([128, 4], f32, tag="stats2g")
            istd2 = small.tile([128, 2], f32, tag="istd2")
            shift2 = small.tile([128, 2], f32, tag="shift2")
            ones_k = small.tile([128, 1], f32r, tag="ones_k")
            ones_m = small.tile([1, 128], f32r, tag="ones_m")
            qloc = persist.tile([128, 2 * R], f32r, tag="qloc")

            eps_sb = small.tile([128, 1], f32, tag="eps")
            ones_k32 = small.tile([128, 1], f32, tag="ones_k32")
            ones_m32 = small.tile([1, 128], f32, tag="ones_m32")
            nc.sync.dma_start(gb_sb[:], gb.ap()[:])
            nc.vector.memset(ones_k32[:], 1.0)
            nc.vector.memset(ones_m32[:], 1.0)
            nc.vector.tensor_copy(ones_k[:], ones_k32[:])
            nc.vector.tensor_copy(ones_m[:], ones_m32[:])
            nc.vector.memset(eps_sb[:], BN_EPS)

            with tc.tile_pool(name="w1", bufs=3) as w1p, \
                 tc.tile_pool(name="zy", bufs=16) as zyp, \
                 tc.tile_pool(name="xx", bufs=16) as xp, \
                 tc.tile_pool(name="w2", bufs=1) as w2p, \
                 tc.tile_pool(name="uu", bufs=2) as up, \
                 tc.tile_pool(name="ph", bufs=2) as php, \
                 tc.tile_pool(name="sq", bufs=2) as sqp, \
                 tc.tile_pool(name="scr", bufs=2) as scrp, \
                 tc.tile_pool(name="nrm", bufs=2) as nrmp, \
                 tc.tile_pool(name="ps13", bufs=4, space="PSUM") as pp13, \
                 tc.tile_pool(name="psn", bufs=2, space="PSUM") as ppn, \
                 tc.tile_pool(name="psb", bufs=2, space="PSUM") as ppb:

                # ---- load z^T (resident) and W2 ----
                zts = []
                for k in range(KT):
                    zt = zyp.tile([128, 2 * R], f32r, tag="zy")
                    nc.sync.dma_start(zt[:], zT.ap()[128 * k:128 * (k + 1), :])
                    zts.append(zt)
                w2sb = w2p.tile([128, KT, 128], f32r, tag="w2")
                nc.sync.dma_start(
                    w2sb[:], W2c.ap().rearrange("(k p) c -> p k c", p=128))

                # ---- phase 1: X^T = (z @ W1.T)^T tiles + BN1 partial sums ----
                xs = []
                for j in range(KT):
                    w1j = w1p.tile([128, KT, 128], f32r, tag="w1")
                    nc.sync.dma_start(
                        w1j[:], W1c.ap()[j].rearrange("(k p) c -> p k c", p=128))
                    xj = xp.tile([128, 2 * R], f32, tag="xx")
                    xs.append(xj)
                    for n in range(2):
                        ps = pp13.tile([128, R], f32, tag="ps13")
                        for k in range(KT):
                            nc.tensor.matmul(
                                ps[:], w1j[:, k, :], zts[k][:, R * n:R * (n + 1)],
                                start=(k == 0), stop=(k == KT - 1))
                        # copy PSUM->SBUF with per-partition row-sum, and
                        # squared row-sum via ACT
                        nc.vector.tensor_scalar(
                            xj[:, R * n:R * (n + 1)], ps[:], 0.0, 0.0,
                            ALU.add, ALU.add,
                            accum_out=stats1[:, 2 * KT * n + j:2 * KT * n + j + 1])
                        scr = scrp.tile([128, R], f32, tag="scr")
                        nc.scalar.activation(
                            scr[:], ps[:], AF.Square,
                            accum_out=stats1[:, 2 * KT * n + KT + j:
                                             2 * KT * n + KT + j + 1])

                if dbg:
                    nc.sync.dma_start(dbg_t["x0"].ap().bitcast(f32)[:], xs[0][:])

                # ---- AllReduce BN1 stats ----
                nc.sync.dma_start(ar1_in.ap()[:], stats1[:])
                nc.gpsimd.collective_compute(
                    "AllReduce", ALU.add, replica_groups=rg,
                    ins=[ar1_in.ap()], outs=[ar1_out.ap()])
                nc.sync.dma_start(stats1_g[:], ar1_out.ap()[:])

                # ---- BN1 coefficients ----
                # stats1 layout: [sum_z1(16) | sumsq_z1(16) | sum_z2(16) | sumsq_z2(16)]
                mean1 = small.tile([128, 2 * KT], f32, tag="mean1")
                var1 = small.tile([128, 2 * KT], f32, tag="var1")
                tmp1 = small.tile([128, 2 * KT], f32, tag="tmp1")
                # mean = sum/N ; e2 = sumsq/N  (strided views: z1 cols 0:16, z2 cols 32:48)
                for zi in range(2):
                    sl_sum = stats1_g[:, 2 * KT * zi:2 * KT * zi + KT]
                    sl_sq = stats1_g[:, 2 * KT * zi + KT:2 * KT * zi + 2 * KT]
                    nc.vector.tensor_scalar_mul(
                        mean1[:, KT * zi:KT * (zi + 1)], sl_sum, 1.0 / N)
                    nc.vector.tensor_scalar_mul(
                        var1[:, KT * zi:KT * (zi + 1)], sl_sq, 1.0 / N)
                nc.vector.tensor_mul(tmp1[:], mean1[:], mean1[:])
                nc.vector.tensor_sub(var1[:], var1[:], tmp1[:])
                # istd = 1/sqrt(var+eps)
                nc.scalar.activation(tmp1[:], var1[:], AF.Sqrt, bias=eps_sb[:])
                nc.vector.reciprocal(var1[:], tmp1[:])  # var1 := istd
                for zi in range(2):
                    sl = slice(KT * zi, KT * (zi + 1))
                    nc.vector.tensor_mul(scale1[:, sl], var1[:, sl], gb_sb[:, 0:KT])
                nc.vector.tensor_mul(tmp1[:], mean1[:], scale1[:])
                for zi in range(2):
                    sl = slice(KT * zi, KT * (zi + 1))
                    nc.vector.tensor_sub(shift1[:, sl], gb_sb[:, KT:2 * KT],
                                         tmp1[:, sl])

                if dbg:
                    nc.sync.dma_start(dbg_t["stats1g"].ap()[:], stats1_g[:])
                    nc.sync.dma_start(dbg_t["scale1"].ap()[:], scale1[:])
                    nc.sync.dma_start(dbg_t["shift1"].ap()[:], shift1[:])

                # ---- phase 2: Y = relu(X*scale + shift)  (f32r) ----
                ys = []
                for j in range(KT):
                    yj = zyp.tile([128, 2 * R], f32r, tag="zy")
                    ys.append(yj)
                    for zi in range(2):
                        nc.scalar.activation(
                            yj[:, R * zi:R * (zi + 1)],
                            xs[j][:, R * zi:R * (zi + 1)], AF.Relu,
                            scale=scale1[:, KT * zi + j:KT * zi + j + 1],
                            bias=shift1[:, KT * zi + j:KT * zi + j + 1])

                if dbg:
                    nc.sync.dma_start(dbg_t["y0"].ap().bitcast(f32r)[:], ys[0][:])

                # ---- phase 3: U^T = (Y @ W2.T)^T + BN2 partial sums ----
                us = []
                for n in range(2):
                    psu = pp13.tile([128, R], f32, tag="ps13")
                    for j in range(KT):
                        nc.tensor.matmul(
                            psu[:], w2sb[:, j, :], ys[j][:, R * n:R * (n + 1)],
                            start=(j == 0), stop=(j == KT - 1))
                    un = up.tile([128, R], f32, tag="uu")
                    us.append(un)
                    nc.vector.tensor_scalar(
                        un[:], psu[:], 0.0, 0.0, ALU.add, ALU.add,
                        accum_out=stats2[:, n:n + 1])
                    scr = scrp.tile([128, R], f32, tag="scr")
                    nc.scalar.activation(
                        scr[:], psu[:], AF.Square,
                        accum_out=stats2[:, 2 + n:3 + n])

                if dbg:
                    nc.sync.dma_start(dbg_t["u0"].ap()[:], us[0][:])

                # ---- AllReduce BN2 stats ----
                nc.sync.dma_start(ar2_in.ap()[:], stats2[:])
                nc.gpsimd.collective_compute(
                    "AllReduce", ALU.add, replica_groups=rg,
                    ins=[ar2_in.ap()], outs=[ar2_out.ap()])
                nc.sync.dma_start(stats2_g[:], ar2_out.ap()[:])

                # ---- BN2 coefficients (affine=False) ----
                mean2 = small.tile([128, 2], f32, tag="mean2")
                var2 = small.tile([128, 2], f32, tag="var2")
                tmp2 = small.tile([128, 2], f32, tag="tmp2")
                nc.vector.tensor_scalar_mul(mean2[:], stats2_g[:, 0:2], 1.0 / N)
                nc.vector.tensor_scalar_mul(var2[:], stats2_g[:, 2:4], 1.0 / N)
                nc.vector.tensor_mul(tmp2[:], mean2[:], mean2[:])
                nc.vector.tensor_sub(var2[:], var2[:], tmp2[:])
                nc.scalar.activation(tmp2[:], var2[:], AF.Sqrt, bias=eps_sb[:])
                nc.vector.reciprocal(istd2[:], tmp2[:])
                nc.vector.tensor_mul(tmp2[:], mean2[:], istd2[:])
                nc.vector.tensor_scalar_mul(shift2[:], tmp2[:], -1.0)

                # ---- phase 4: P = BN2(U); qhat = P/||P||; stage for AllGather ----
                for zi in range(2):
                    ph = php.tile([128, R], f32, tag="ph")
                    nc.vector.tensor_scalar(
                        ph[:], us[zi][:], istd2[:, zi:zi + 1],
                        shift2[:, zi:zi + 1], ALU.mult, ALU.add)
                    sq = sqp.tile([128, R], f32r, tag="sq")
                    nc.scalar.activation(sq[:], ph[:], AF.Square)
                    n2 = ppn.tile([1, R], f32, tag="psn")
                    nc.tensor.matmul(n2[:], ones_k[:], sq[:], start=True, stop=True)
                    nrm = nrmp.tile([1, R], f32, tag="nrm")
                    nc.scalar.activation(nrm[:], n2[:], AF.Sqrt)
                    rinv = nrmp.tile([1, R], f32r, tag="rinv")
                    with nc.allow_low_precision(reason="f32r rounding of 1/||p|| is intentional"):
                        nc.vector.reciprocal(rinv[:], nrm[:])
                    rb = ppb.tile([128, R], f32, tag="psb")
                    nc.tensor.matmul(rb[:], ones_m[:], rinv[:], start=True, stop=True)
                    nc.vector.tensor_mul(qloc[:, R * zi:R * (zi + 1)], ph[:], rb[:])

                nc.sync.dma_start(ag_in.ap().bitcast(f32r)[:], qloc[:])
                if dbg:
                    nc.sync.dma_start(dbg_t["stats2g"].ap()[:], stats2_g[:])
                    nc.sync.dma_start(dbg_t["qloc"].ap().bitcast(f32r)[:], qloc[:])

            # phase 1-4 pools released here
            nc.gpsimd.collective_compute(
                "AllGather", ALU.bypass, replica_groups=rg,
                ins=[ag_in.ap()], outs=[ag_out.ap()])

            with tc.tile_pool(name="qf", bufs=1) as qfp, \
                 tc.tile_pool(name="stage", bufs=8) as stp, \
                 tc.tile_pool(name="ps5", bufs=6, space="PSUM") as pp5:
                qfull = qfp.tile([128, 2 * N], f32r, tag="qf")
                for c in range(NCORES):
                    nc.sync.dma_start(
                        qfull[:, 2 * R * c:2 * R * (c + 1)],
                        ag_out.ap().bitcast(f32r)[128 * c:128 * (c + 1), :])

                if dbg:
                    nc.sync.dma_start(dbg_t["qfull"].ap().bitcast(f32r)[:], qfull[:])

                # ---- phase 5: row-block of the 2N x 2N score matrix ----
                for t in range(8):
                    lhs = qloc[:, 128 * t:128 * (t + 1)]
                    out_dram = out_top if t < 4 else out_bot
                    row0 = 128 * (t % 4)
                    for b in range(2 * NCORES):
                        cb, zb = b // 2, b % 2
                        gcol = N * zb + R * cb
                        pss = pp5.tile([128, R], f32, tag="ps5")
                        nc.tensor.matmul(
                            pss[:], lhs, qfull[:, R * b:R * (b + 1)],
                            start=True, stop=True)
                        ob = stp.tile([128, R], f32, tag="stage")
                        if b % 2 == 0:
                            nc.vector.tensor_scalar_mul(ob[:], pss[:], TEMP_SCALE)
                        else:
                            nc.scalar.mul(ob[:], pss[:], TEMP_SCALE)
                        nc.sync.dma_start(
                            out_dram.ap()[row0:row0 + 128, gcol:gcol + R], ob[:])

    nc.compile()
    return nc


def _get_nc():
    if "nc" not in _CACHE:
        _CACHE["nc"] = _build()
    return _CACHE["nc"]


def kernel(z1, z2, W1, gamma1, beta1, W2):
    z1 = np.asarray(z1, dtype=np.float32)
    z2 = np.asarray(z2, dtype=np.float32)
    W1 = np.asarray(W1, dtype=np.float32)
    W2 = np.asarray(W2, dtype=np.float32)
    gamma1 = np.asarray(gamma1, dtype=np.float32)
    beta1 = np.asarray(beta1, dtype=np.float32)

    W1T = np.ascontiguousarray(W1.T)                       # [k, j]
    W1c = np.ascontiguousarray(
        W1T.reshape(D, KT, 128).transpose(1, 0, 2))        # [j, k, 128]
    W2c = np.ascontiguousarray(W2.T)                       # [D, 128]
    gb = np.concatenate([gamma1.reshape(KT, 128).T,
                         beta1.reshape(KT, 128).T], axis=1).astype(np.float32)
    gb = np.ascontiguousarray(gb)                          # [128, 32]

    in_maps = []
    for c in range(NCORES):
        rs = slice(R * c, R * (c + 1))
        zTc = np.concatenate([z1[rs].T, z2[rs].T], axis=1)  # [D, 1024]
        in_maps.append({
            "zT": np.ascontiguousarray(zTc),
            "W1c": W1c, "W2c": W2c, "gb": gb,
        })

    nc = _get_nc()
    res = run_bass_kernel_spmd(nc, in_maps, core_ids=list(range(NCORES)))

    scores = np.empty((2 * N, 2 * N), dtype=np.float32)
    for c in range(NCORES):
        scores[R * c:R * (c + 1), :] = res.results[c]["out_top"]
        scores[N + R * c:N + R * (c + 1), :] = res.results[c]["out_bot"]
    idx = np.arange(N)
    scores[idx, idx] = -np.inf
    scores[N + idx, N + idx] = -np.inf
    targets = np.arange(2 * N, dtype=np.int32)
    return scores, targets
